# revision 1
# baseline (speedup 1.0000x reference)
"""ChessBoardAttention Trainium2 kernel.

Full inputs -> full output. The 32 independent (batch, chessboard-offset)
attention problems are sharded 4-per-core across 8 NeuronCores; the
chessboard gather/scatter is pure data movement done host-side as part of
sharding.

Per-core device kernel, per problem (x_off: [64, 2304] f32):
  q/k = relu(Wqk @ x + b)            [8, L]   (bias via ones-row in x)
  vT  = relu(x_chunk.T @ Wv.T + bv)  [L, 64]  (computed transposed, 128-row chunks)
  S_T[m, l] = k[:,m-chunk].T @ q     scores computed TRANSPOSED so that the
                                     AV contraction runs over PSUM partitions
  P_T = exp(S_T)                     (no max-subtraction needed: s in [0, ~20])
  AV: out[c, l] = sum_m vT_aug[m, c] P_T[m, l], where vT_aug column 64 is
      filled with 1/gamma so row 64 of the accumulator is Z/gamma, making
      the final normalize out * recip(Z/gamma) = gamma * softmax @ v.
  out = AV * recip + x_off           (residual)
"""

import numpy as np

import concourse.bass as bass
import concourse.tile as tile
from concourse import mybir
from concourse.bass_utils import run_bass_kernel_spmd

F32 = mybir.dt.float32
F32R = mybir.dt.float32r
AT = mybir.AluOpType

B, C, H, W = 2, 64, 192, 192
C8 = 8
HQ, WQ = H // 4, W // 4
L = HQ * WQ            # 2304
NPROB = 4              # problems per core
NCORES = 8
NM = L // 128          # 18 m-chunks of 128
LBLOCKS = [(0, 512), (512, 512), (1024, 512), (1536, 512), (2048, 256)]
VS = C + 1             # v-chunk stride in vT_sb (64 channels + 1/gamma col)
SGRP = 3               # m-chunks per score-psum group (3 banks)
NGRP = NM // SGRP      # 6 groups


def split_drain_waits(nc, keep=1):
    """This walrus build rejects instructions carrying more than a couple of
    sem-waits. Move excess waits onto single-wait DRAIN instructions inserted
    just before the offender on the same engine (drains with one wait are
    known-good through codegen)."""
    for f in nc.m.functions:
        for bb in f.blocks:
            insts = bb.instructions
            idx = 0
            while idx < len(insts):
                i = insts[idx]
                si = i.sync_info
                lim = keep
                if si is not None and si.on_wait and len(si.on_wait) > lim:
                    waits = list(si.on_wait)
                    si.on_wait = waits[-lim:]
                    for k, wt in enumerate(waits[:-lim]):
                        d = mybir.InstDrain(
                            name=f"{i.name}_wsplit{k}", ins=[], outs=[],
                            bass_is_fusable=False,
                        )
                        d.engine = i.engine
                        d.sync_info = mybir.SyncInfo(on_wait=[wt], on_update=[])
                        nc.register_instruction(d)
                        insts.insert(idx, d)
                        idx += 1
                idx += 1


def build_module():
    nc = bass.Bass("TRN2", target_bir_lowering=False, debug=False,
                   enable_asserts=False)
    xoffs = nc.dram_tensor("xoffs", [NPROB, C, L], F32, kind="ExternalInput").ap()
    wqk = nc.dram_tensor("wqk", [C + 1, 40], F32, kind="ExternalInput").ap()
    wv = nc.dram_tensor("wv", [C + 1, C], F32, kind="ExternalInput").ap()
    invg_col = nc.dram_tensor("invg_col", [128, NM], F32, kind="ExternalInput").ap()
    out_d = nc.dram_tensor("out", [NPROB, C, L], F32, kind="ExternalOutput").ap()

    with tile.TileContext(nc) as tc:
        with (
            tc.tile_pool(name="singles", bufs=1) as singles,
            tc.tile_pool(name="io", bufs=2) as io,
            tc.tile_pool(name="qk", bufs=2) as qkp,
            tc.tile_pool(name="vt", bufs=2) as vtp,
            tc.tile_pool(name="pt", bufs=2) as ptp,
            tc.tile_pool(name="small", bufs=2) as smallp,
            tc.tile_pool(name="ps_s", bufs=2, space="PSUM") as ps_s_p,
            tc.tile_pool(name="ps_o", bufs=1, space="PSUM") as ps_o_p,
            tc.tile_pool(name="ps_proj", bufs=1, space="PSUM") as ps_proj_p,
            tc.tile_pool(name="dram", bufs=2, space="DRAM") as dramp,
        ):
            wqk_sb = singles.tile([C + 1, 40], F32)
            nc.sync.dma_start(out=wqk_sb, in_=wqk)
            wv_sb = singles.tile([C + 1, C], F32)
            nc.sync.dma_start(out=wv_sb, in_=wv)

            for p in range(NPROB):
                # ---- load x (+ ones row for the bias trick) ----
                x_sb = io.tile([C + 1, L], F32, tag="x")
                nc.sync.dma_start(out=x_sb[0:C, :], in_=xoffs[p])
                nc.gpsimd.memset(x_sb[C : C + 1, :], 1.0)

                # ---- q/k projection: [16, L] = wqk.T @ x_aug, relu ----
                q_sb = qkp.tile([C8, L], F32R, tag="q")
                k_sb = qkp.tile([C8, L], F32R, tag="k")
                for st, w in LBLOCKS:
                    ps = ps_proj_p.tile([128, 512], F32, tag="proj")
                    nc.tensor.matmul(
                        ps[:40, :w], lhsT=wqk_sb, rhs=x_sb[:, st : st + w],
                        start=True, stop=True,
                    )
                    nc.vector.tensor_scalar_max(
                        out=q_sb[:, st : st + w], in0=ps[0:C8, :w], scalar1=0.0)
                    nc.vector.tensor_scalar_max(
                        out=k_sb[:, st : st + w], in0=ps[32:40, :w], scalar1=0.0)

                # ---- v projection, transposed: vT[m, c] in 128-row chunks ----
                vT_sb = vtp.tile([128, NM * VS], F32R, tag="vt")
                vT3 = vT_sb.rearrange("p (n c) -> p n c", c=VS)
                invg_sb = smallp.tile([128, NM], F32, tag="invg")
                nc.sync.dma_start(out=invg_sb, in_=invg_col)
                nc.vector.tensor_copy(vT3[:, :, C], invg_sb)
                for g in range(3):
                    cnt = 8 if g < 2 else NM - 16
                    ps = ps_proj_p.tile([128, 512], F32, tag="proj")
                    for j in range(cnt):
                        mc = g * 8 + j
                        nc.tensor.matmul(
                            ps[:, j * C : (j + 1) * C],
                            lhsT=x_sb[:, mc * 128 : (mc + 1) * 128],
                            rhs=wv_sb, start=True, stop=True,
                        )
                    ps3 = ps.rearrange("p (n c) -> p n c", c=C)
                    nc.vector.tensor_scalar_max(
                        out=vT3[:, g * 8 : g * 8 + cnt, 0:C],
                        in0=ps3[:, 0:cnt, :], scalar1=0.0)

                # ---- attention over l-blocks ----
                av_sb = io.tile([C + 1, L], F32, tag="av")
                for st, w in LBLOCKS:
                    pT_sb = ptp.tile([128, NM * 512], F32R, tag="pt")
                    pT3 = pT_sb.rearrange("p (n c) -> p n c", c=512)
                    for g in range(NGRP):
                        ps_s = ps_s_p.tile([128, SGRP * 512], F32, tag="s")
                        for j in range(SGRP):
                            mc = g * SGRP + j
                            nc.tensor.matmul(
                                ps_s[:, j * 512 : j * 512 + w],
                                lhsT=k_sb[:, mc * 128 : (mc + 1) * 128],
                                rhs=q_sb[:, st : st + w],
                                start=True, stop=True,
                            )
                        ps_s3 = ps_s.rearrange("p (n c) -> p n c", c=512)
                        nc.scalar.activation(
                            out=pT3[:, g * SGRP : (g + 1) * SGRP, :w],
                            in_=ps_s3[:, :, :w],
                            func=mybir.ActivationFunctionType.Exp,
                        )
                    ps_o = ps_o_p.tile([C + 1, 512], F32, tag="o")
                    for mc in range(NM):
                        nc.tensor.matmul(
                            ps_o[:, :w],
                            lhsT=vT3[:, mc, :],
                            rhs=pT3[:, mc, :w],
                            start=(mc == 0), stop=(mc == NM - 1),
                        )
                    nc.vector.tensor_copy(av_sb[:, st : st + w], ps_o[:, :w])

                # ---- normalize (row C of av_sb is Z/gamma), scale, residual ----
                nc.vector.reciprocal(
                    out=av_sb[C : C + 1, :], in_=av_sb[C : C + 1, :])
                dram_rec = dramp.tile([1, L], F32, tag="drec")
                nc.sync.dma_start(out=dram_rec, in_=av_sb[C : C + 1, :])
                rec_rep = smallp.tile([C, L], F32, tag="recrep")
                rec_b = bass.AP(
                    tensor=dram_rec.tensor, offset=dram_rec.offset,
                    ap=[[0, C]] + list(dram_rec.ap)[1:])
                nc.sync.dma_start(out=rec_rep, in_=rec_b)
                nc.vector.tensor_tensor(
                    out=av_sb[0:C, :], in0=av_sb[0:C, :], in1=rec_rep, op=AT.mult)
                nc.gpsimd.tensor_tensor(
                    out=av_sb[0:C, :], in0=av_sb[0:C, :], in1=x_sb[0:C, :], op=AT.add)
                nc.sync.dma_start(out=out_d[p], in_=av_sb[0:C, :])

    split_drain_waits(nc)
    return nc


_NC = None


def _get_nc():
    global _NC
    if _NC is None:
        _NC = build_module()
    return _NC


def make_in_maps(x, Wq, bq, Wk, bk, Wv, bv, gamma):
    x = np.asarray(x, np.float32)
    xoff = (
        x.reshape(B, C, HQ, 4, WQ, 4)
        .transpose(0, 3, 5, 1, 2, 4)
        .reshape(B * 16, C, L)
    )
    wqk = np.zeros((C + 1, 40), np.float32)   # q -> psum parts 0-7, k -> 32-39
    wqk[:C, 0:C8] = np.asarray(Wq).T
    wqk[C, 0:C8] = np.asarray(bq)
    wqk[:C, 32:40] = np.asarray(Wk).T
    wqk[C, 32:40] = np.asarray(bk)
    wv = np.concatenate([np.asarray(Wv).T, np.asarray(bv)[None, :]], 0).astype(
        np.float32
    )                                         # [65, 64]
    with np.errstate(divide="ignore"):
        invg = np.float32(1.0) / np.float32(np.asarray(gamma).reshape(-1)[0])
    invg_col = np.full((128, NM), invg, np.float32)
    in_maps = []
    for c in range(NCORES):
        in_maps.append(
            {
                "xoffs": np.ascontiguousarray(xoff[c * NPROB : (c + 1) * NPROB]),
                "wqk": wqk,
                "wv": wv,
                "invg_col": invg_col,
            }
        )
    return in_maps


def unshard(results):
    outp = np.concatenate([results[c]["out"] for c in range(NCORES)], 0)
    return (
        outp.reshape(B, 4, 4, C, HQ, WQ)
        .transpose(0, 3, 4, 1, 5, 2)
        .reshape(B, C, H, W)
        .astype(np.float32)
    )


def kernel(**inputs):
    nc = _get_nc()
    in_maps = make_in_maps(**inputs)
    res = run_bass_kernel_spmd(nc, in_maps, list(range(NCORES)))
    return unshard(res.results)



# revision 5
# speedup vs baseline: 1.2092x; 1.2092x over previous
"""ChessBoardAttention Trainium2 kernel.

Full inputs -> full output. The 32 independent (batch, chessboard-offset)
attention problems are sharded 4-per-core across 8 NeuronCores; the
chessboard gather/scatter is pure data movement done host-side as part of
sharding.

Per-core device kernel, per problem (x_off: [64, 2304]), all matmul
operands bf16:
  qk  = relu(Wqk @ x + b)            [40, L]  one relu per l-block
                                     (q rows 0-7, k rows 32-39);
                                     GpSimd copies k to a base-0 tile
  vT  = relu(x_chunk.T @ Wv.T + bv)  [128-chunks, 65]  col 64 = 1/gamma
  S_T[m, l] = k[:,m-chunk].T @ q     scores TRANSPOSED, 2-m-chunk psum groups
  P_T = exp(S_T)                     split between Act (exact Exp) and DVE
                                     (Schraudolph: bf16 bits = rint(a*s+b)
                                     via fp32->int16 convert, bitcast)
  AV (transposed): out_T[l, c] = sum_m P_T[m, l] vT[m, c] accumulated over
      18 m-chunks into PSUM [128, 65]; col 64 = Z/gamma.
  out_T = (out_T[:, :64] * (gamma/Z)[l]) + xT   fused scalar_tensor_tensor
  Output written l-major [128, 18*64]; host undoes the transpose.

The AV matmuls of block b are interleaved between the score-matmul groups
of block b+1 (and the projection groups of the next problem) so the PE
never idles while the exp engines drain score psum groups.
"""

import numpy as np
import ml_dtypes

import concourse.bass as bass
import concourse.tile as tile
from concourse import mybir
from concourse.bass_utils import run_bass_kernel_spmd

F32 = mybir.dt.float32
BF16 = mybir.dt.bfloat16
I16 = mybir.dt.int16
AT = mybir.AluOpType
AF = mybir.ActivationFunctionType

B, C, H, W = 2, 64, 192, 192
C8 = 8
HQ, WQ = H // 4, W // 4
L = HQ * WQ            # 2304
NPROB = 4              # problems per core
NCORES = 8
NM = L // 128          # 18 m-chunks of 128
LBLOCKS = [(0, 512), (512, 512), (1024, 512), (1536, 512), (2048, 256)]
VS = C + 1             # v-chunk stride in vT_sb (64 channels + 1/gamma col)
NGRP = NM // 2         # 9 score psum groups of 2 m-chunks per l-block

# Schraudolph exp for bf16: bits16 = rint(A16*s + B16); bitcast int16->bf16.
A16 = float(128.0 / np.log(2.0))
B16 = float(127.0 * 128.0 - 7.4)

# exp engine per score group: Act 5, DVE 4 per block of 9
EXP_ENG = ["A", "D", "A", "D", "A", "D", "A", "D", "A"]


def split_drain_waits(nc, keep=1):
    """This walrus build rejects instructions carrying more than a couple of
    sem-waits. Move excess waits onto single-wait DRAIN instructions inserted
    just before the offender on the same engine (drains with one wait are
    known-good through codegen)."""
    for f in nc.m.functions:
        for bb in f.blocks:
            insts = bb.instructions
            idx = 0
            while idx < len(insts):
                i = insts[idx]
                si = i.sync_info
                lim = keep
                if si is not None and si.on_wait and len(si.on_wait) > lim:
                    waits = list(si.on_wait)
                    si.on_wait = waits[-lim:]
                    for k, wt in enumerate(waits[:-lim]):
                        d = mybir.InstDrain(
                            name=f"{i.name}_wsplit{k}", ins=[], outs=[],
                            bass_is_fusable=False,
                        )
                        d.engine = i.engine
                        d.sync_info = mybir.SyncInfo(on_wait=[wt], on_update=[])
                        nc.register_instruction(d)
                        insts.insert(idx, d)
                        idx += 1
                idx += 1


class AvQueue:
    """Pending AV matmuls for one finished l-block, drained a few at a time
    between later PE work so the tensor engine never stalls on exp."""

    def __init__(self, nc, work_pool, small_pool, pT3, st, w, vT3, out_sb,
                 xT_sb, out_dma=None):
        self.nc = nc
        self.small = small_pool
        self.pT3, self.st, self.w = pT3, st, w
        self.vT3, self.out_sb, self.xT_sb = vT3, out_sb, xT_sb
        self.out_dma = out_dma
        self.nsub = w // 128
        self.ps_av = work_pool.tile([128, 512], F32, tag="work")
        self.items = [(sub, mc) for sub in range(self.nsub) for mc in range(NM)]
        self.pos = 0

    def drain(self, n):
        nc = self.nc
        end = min(self.pos + n, len(self.items))
        for i in range(self.pos, end):
            sub, mc = self.items[i]
            nc.tensor.matmul(
                self.ps_av[:, sub * VS : sub * VS + VS],
                lhsT=self.pT3[:, mc, sub * 128 : (sub + 1) * 128],
                rhs=self.vT3[:, mc, :],
                start=(mc == 0), stop=(mc == NM - 1),
            )
        self.pos = end

    def finish(self):
        nc = self.nc
        self.drain(len(self.items))
        rec = self.small.tile([128, 4], F32, tag="rec")
        zview = bass.AP(
            tensor=self.ps_av.tensor, offset=self.ps_av.offset + C,
            ap=[list(self.ps_av.ap)[0], [VS, self.nsub]])
        nc.vector.reciprocal(out=rec[:, 0 : self.nsub], in_=zview)
        for sub in range(self.nsub):
            ci = self.st // 128 + sub
            nc.vector.scalar_tensor_tensor(
                out=self.out_sb[:, ci * C : (ci + 1) * C],
                in0=self.ps_av[:, sub * VS : sub * VS + C],
                scalar=rec[:, sub : sub + 1],
                in1=self.xT_sb[:, ci * C : (ci + 1) * C],
                op0=AT.mult, op1=AT.add,
            )
        if self.out_dma is not None:
            nc.sync.dma_start(out=self.out_dma, in_=self.out_sb)


def build_module():
    nc = bass.Bass("TRN2", target_bir_lowering=False, debug=False,
                   enable_asserts=False)
    xoffs = nc.dram_tensor("xoffs", [NPROB, C, L], BF16, kind="ExternalInput").ap()
    xT_d = nc.dram_tensor("xT", [NPROB, 128, NM * C], F32, kind="ExternalInput").ap()
    wqk = nc.dram_tensor("wqk", [C + 1, 40], BF16, kind="ExternalInput").ap()
    wv = nc.dram_tensor("wv", [C + 1, C], BF16, kind="ExternalInput").ap()
    invg_col = nc.dram_tensor("invg_col", [128, NM], BF16, kind="ExternalInput").ap()
    out_d = nc.dram_tensor("out", [NPROB, 128, NM * C], F32, kind="ExternalOutput").ap()

    with tile.TileContext(nc) as tc:
        with (
            tc.tile_pool(name="singles", bufs=1) as singles,
            tc.tile_pool(name="io", bufs=2) as io,
            tc.tile_pool(name="qk", bufs=2) as qkp,
            tc.tile_pool(name="vt", bufs=2) as vtp,
            tc.tile_pool(name="pt", bufs=2) as ptp,
            tc.tile_pool(name="small", bufs=2) as smallp,
            tc.tile_pool(name="ps_s", bufs=2, space="PSUM") as ps_sp,
            tc.tile_pool(name="work", bufs=4, space="PSUM") as workp,
        ):
            wqk_sb = singles.tile([C + 1, 40], BF16)
            nc.sync.dma_start(out=wqk_sb, in_=wqk)
            wv_sb = singles.tile([C + 1, C], BF16)
            nc.sync.dma_start(out=wv_sb, in_=wv)
            invg_sb = singles.tile([128, NM], BF16)
            nc.sync.dma_start(out=invg_sb, in_=invg_col)

            av_q = None

            def drain(n):
                if av_q is not None:
                    av_q.drain(n)

            def emit_load(p):
                x_sb = io.tile([C + 1, L], BF16, tag="x")
                nc.sync.dma_start(out=x_sb[0:C, :], in_=xoffs[p])
                nc.gpsimd.memset(x_sb[C : C + 1, :], 1.0)
                xT_sb = io.tile([128, NM * C], F32, tag="xt")
                nc.sync.dma_start(out=xT_sb, in_=xT_d[p])
                out_sb = io.tile([128, NM * C], F32, tag="out")
                return x_sb, xT_sb, out_sb

            def emit_proj(p, x_sb):
                # q/k projection: one relu per block into qk_sb [40, L];
                # GpSimd copies the k rows to base-0 k0_sb.
                qk_sb = qkp.tile([40, L], BF16, tag="qk")
                k0_sb = qkp.tile([C8, L], BF16, tag="k0")
                for st, w in LBLOCKS:
                    ps = workp.tile([128, 512], F32, tag="work")
                    nc.tensor.matmul(
                        ps[:40, :w], lhsT=wqk_sb, rhs=x_sb[:, st : st + w],
                        start=True, stop=True,
                    )
                    drain(9)
                    nc.scalar.activation(
                        out=qk_sb[:, st : st + w], in_=ps[:40, :w], func=AF.Relu)
                    nc.gpsimd.tensor_copy(
                        k0_sb[:, st : st + w], qk_sb[32:40, st : st + w])
                # v projection, transposed, 128-row chunks
                vT_sb = vtp.tile([128, NM * VS], BF16, tag="vt")
                vT3 = vT_sb.rearrange("p (n c) -> p n c", c=VS)
                nc.gpsimd.tensor_copy(vT3[:, :, C], invg_sb)
                for g in range(3):
                    cnt = 8 if g < 2 else NM - 16
                    ps = workp.tile([128, 512], F32, tag="work")
                    for j in range(cnt):
                        mc = g * 8 + j
                        nc.tensor.matmul(
                            ps[:, j * C : (j + 1) * C],
                            lhsT=x_sb[:, mc * 128 : (mc + 1) * 128],
                            rhs=wv_sb, start=True, stop=True,
                        )
                    drain(9)
                    ps3 = ps.rearrange("p (n c) -> p n c", c=C)
                    nc.scalar.activation(
                        out=vT3[:, g * 8 : g * 8 + cnt, 0:C],
                        in_=ps3[:, 0:cnt, :], func=AF.Relu)
                return qk_sb, k0_sb, vT3

            x_sb, xT_sb, out_sb = emit_load(0)
            qk_sb, k0_sb, vT3 = emit_proj(0, x_sb)

            for p in range(NPROB):
                for st, w in LBLOCKS:
                    pT_sb = ptp.tile([128, NM * 512], BF16, tag="pt")
                    pT3 = pT_sb.rearrange("p (n c) -> p n c", c=512)
                    for g in range(NGRP):
                        ps_s = ps_sp.tile([128, 1024], F32, tag="s")
                        for j in range(2):
                            mc = 2 * g + j
                            nc.tensor.matmul(
                                ps_s[:, j * 512 : j * 512 + w],
                                lhsT=k0_sb[:, mc * 128 : (mc + 1) * 128],
                                rhs=qk_sb[0:C8, st : st + w],
                                start=True, stop=True,
                            )
                        drain(8)
                        ps_s3 = ps_s.rearrange("p (n c) -> p n c", c=512)
                        if EXP_ENG[g] == "A":
                            nc.scalar.activation(
                                out=pT3[:, 2 * g : 2 * g + 2, :w],
                                in_=ps_s3[:, :, :w], func=AF.Exp)
                        else:
                            nc.vector.tensor_scalar(
                                out=pT3[:, 2 * g : 2 * g + 2, :w].bitcast(I16),
                                in0=ps_s3[:, :, :w], scalar1=A16, scalar2=B16,
                                op0=AT.mult, op1=AT.add)
                    if av_q is not None:
                        av_q.finish()
                    is_last = (st, w) == LBLOCKS[-1]
                    av_q = AvQueue(
                        nc, workp, smallp, pT3, st, w, vT3, out_sb, xT_sb,
                        out_dma=out_d[p] if is_last else None)
                if p + 1 < NPROB:
                    nx, nxT, nout = emit_load(p + 1)
                    nqk, nk0, nvT3 = emit_proj(p + 1, nx)
                    x_sb, xT_sb, out_sb = nx, nxT, nout
                    qk_sb, k0_sb, vT3 = nqk, nk0, nvT3
            av_q.finish()

    split_drain_waits(nc)
    return nc


_NC = None


def _get_nc():
    global _NC
    if _NC is None:
        _NC = build_module()
    return _NC


def make_in_maps(x, Wq, bq, Wk, bk, Wv, bv, gamma):
    bf = ml_dtypes.bfloat16
    x = np.asarray(x, np.float32)
    xoff = (
        x.reshape(B, C, HQ, 4, WQ, 4)
        .transpose(0, 3, 5, 1, 2, 4)
        .reshape(B * 16, C, L)
    )
    xoff_bf = np.ascontiguousarray(xoff.astype(bf))
    # transposed residual, chunk-major: [prob, 128, NM*C]
    xT = np.ascontiguousarray(
        xoff.transpose(0, 2, 1)
        .reshape(B * 16, NM, 128, C)
        .transpose(0, 2, 1, 3)
        .reshape(B * 16, 128, NM * C)
    )
    wqk = np.zeros((C + 1, 40), np.float32)   # q -> psum parts 0-7, k -> 32-39
    wqk[:C, 0:C8] = np.asarray(Wq).T
    wqk[C, 0:C8] = np.asarray(bq)
    wqk[:C, 32:40] = np.asarray(Wk).T
    wqk[C, 32:40] = np.asarray(bk)
    wqk = wqk.astype(bf)
    wv = np.concatenate([np.asarray(Wv).T, np.asarray(bv)[None, :]], 0).astype(bf)
    with np.errstate(divide="ignore"):
        invg = np.float32(1.0) / np.float32(np.asarray(gamma).reshape(-1)[0])
    invg_col = np.full((128, NM), invg, np.float32).astype(bf)
    in_maps = []
    for c in range(NCORES):
        sl = slice(c * NPROB, (c + 1) * NPROB)
        in_maps.append(
            {
                "xoffs": np.ascontiguousarray(xoff_bf[sl]),
                "xT": np.ascontiguousarray(xT[sl]),
                "wqk": wqk,
                "wv": wv,
                "invg_col": invg_col,
            }
        )
    return in_maps


def unshard(results):
    outp = np.concatenate([results[c]["out"] for c in range(NCORES)], 0)
    # [32, 128, NM*C] l-minor-transposed -> [32, C, L]
    outp = (
        outp.reshape(B * 16, 128, NM, C)
        .transpose(0, 3, 2, 1)          # [32, C, NM, 128]
        .reshape(B * 16, C, L)
    )
    return (
        outp.reshape(B, 4, 4, C, HQ, WQ)
        .transpose(0, 3, 4, 1, 5, 2)
        .reshape(B, C, H, W)
        .astype(np.float32)
    )


def kernel(**inputs):
    nc = _get_nc()
    in_maps = make_in_maps(**inputs)
    res = run_bass_kernel_spmd(nc, in_maps, list(range(NCORES)))
    return unshard(res.results)


# revision 9
# speedup vs baseline: 1.4411x; 1.1917x over previous
"""ChessBoardAttention Trainium2 kernel.

Full inputs -> full output. The 32 independent (batch, chessboard-offset)
attention problems are sharded 4-per-core across 8 NeuronCores; the
chessboard gather/scatter is pure data movement done host-side as part of
sharding.

Per-core device kernel, per problem (x_off: [64, 2304]), all matmul
operands bf16:
  qk  = relu(Wqk @ x + b)            [40, L]  one relu per l-block
                                     (q rows 0-7, k rows 32-39);
                                     GpSimd copies k to a base-0 tile
  vT  = relu(x_chunk.T @ Wv.T + bv)  [128-chunks, 65]  col 64 = 1/gamma
  S_T[m, l] = k[:,m-chunk].T @ q     scores TRANSPOSED, 2-m-chunk psum groups
  P_T = exp(S_T)                     split between Act (exact Exp) and DVE
                                     (Schraudolph: bf16 bits = rint(a*s+b)
                                     via fp32->int16 convert, bitcast)
  AV (transposed): out_T[l, c] = sum_m P_T[m, l] vT[m, c] accumulated over
      18 m-chunks into PSUM [128, 65]; col 64 = Z/gamma.
  out_T = (out_T[:, :64] * (gamma/Z)[l]) + xT   fused scalar_tensor_tensor
  Output written l-major [128, 18*64]; host undoes the transpose.

The AV matmuls of block b are interleaved between the score-matmul groups
of block b+1 (and the projection groups of the next problem) so the PE
never idles while the exp engines drain score psum groups.
"""

import numpy as np
import ml_dtypes

import concourse.bass as bass
import concourse.tile as tile
from concourse import mybir
from concourse.bass_utils import run_bass_kernel_spmd

F32 = mybir.dt.float32
BF16 = mybir.dt.bfloat16
I16 = mybir.dt.int16
AT = mybir.AluOpType
AF = mybir.ActivationFunctionType

B, C, H, W = 2, 64, 192, 192
C8 = 8
HQ, WQ = H // 4, W // 4
L = HQ * WQ            # 2304
NPROB = 4              # problems per core
NCORES = 8
NM = L // 128          # 18 m-chunks of 128
LBLOCKS = [(0, 512), (512, 512), (1024, 512), (1536, 512), (2048, 256)]
VS = C + 1             # v-chunk stride in vT_sb (64 channels + 1/gamma col)
NGRP = NM // 2         # 9 score psum groups of 2 m-chunks per l-block

# Schraudolph exp for bf16: bits16 = rint(A16*s + B16); bitcast int16->bf16.
A16 = float(128.0 / np.log(2.0))
B16 = float(127.0 * 128.0 - 7.4)

# exp engine per score group, cycled per block: Act ~5.3, DVE ~3.7 of 9
EXP_PATTERNS = [
    ["A", "D", "A", "D", "A", "D", "A", "D", "A"],   # 5A/4D
    ["A", "D", "A", "D", "A", "D", "A", "D", "A"],   # 5A/4D
    ["A", "D", "A", "A", "D", "A", "A", "D", "A"],   # 6A/3D
]


def split_drain_waits(nc, keep=1):
    """This walrus build rejects instructions carrying more than a couple of
    sem-waits. Move excess waits onto single-wait DRAIN instructions inserted
    just before the offender on the same engine (drains with one wait are
    known-good through codegen)."""
    for f in nc.m.functions:
        for bb in f.blocks:
            insts = bb.instructions
            idx = 0
            while idx < len(insts):
                i = insts[idx]
                si = i.sync_info
                lim = keep
                if si is not None and si.on_wait and len(si.on_wait) > lim:
                    waits = list(si.on_wait)
                    si.on_wait = waits[-lim:]
                    for k, wt in enumerate(waits[:-lim]):
                        d = mybir.InstDrain(
                            name=f"{i.name}_wsplit{k}", ins=[], outs=[],
                            bass_is_fusable=False,
                        )
                        d.engine = i.engine
                        d.sync_info = mybir.SyncInfo(on_wait=[wt], on_update=[])
                        nc.register_instruction(d)
                        insts.insert(idx, d)
                        idx += 1
                idx += 1


class AvQueue:
    """Pending AV matmuls for one finished l-block, drained a few at a time
    between later PE work so the tensor engine never stalls on exp."""

    def __init__(self, nc, work_pool, small_pool, pT3, st, w, vT3, out_sb,
                 xT_sb, out_dma=None):
        self.nc = nc
        self.small = small_pool
        self.pT3, self.st, self.w = pT3, st, w
        self.vT3, self.out_sb, self.xT_sb = vT3, out_sb, xT_sb
        self.out_dma = out_dma
        self.nsub = w // 128
        self.ps_av = work_pool.tile([128, 512], F32, tag="work")
        self.items = [(sub, mc) for sub in range(self.nsub) for mc in range(NM)]
        self.pos = 0

    def drain(self, n):
        nc = self.nc
        end = min(self.pos + n, len(self.items))
        for i in range(self.pos, end):
            sub, mc = self.items[i]
            nc.tensor.matmul(
                self.ps_av[:, sub * VS : sub * VS + VS],
                lhsT=self.pT3[:, mc, sub * 128 : (sub + 1) * 128],
                rhs=self.vT3[:, mc, :],
                start=(mc == 0), stop=(mc == NM - 1),
            )
        self.pos = end

    def finish(self):
        nc = self.nc
        self.drain(len(self.items))
        rec = self.small.tile([128, 4], F32, tag="rec")
        zview = bass.AP(
            tensor=self.ps_av.tensor, offset=self.ps_av.offset + C,
            ap=[list(self.ps_av.ap)[0], [VS, self.nsub]])
        nc.vector.reciprocal(out=rec[:, 0 : self.nsub], in_=zview)
        for sub in range(self.nsub):
            ci = self.st // 128 + sub
            nc.vector.scalar_tensor_tensor(
                out=self.out_sb[:, ci * C : (ci + 1) * C],
                in0=self.ps_av[:, sub * VS : sub * VS + C],
                scalar=rec[:, sub : sub + 1],
                in1=self.xT_sb[:, ci * C : (ci + 1) * C],
                op0=AT.mult, op1=AT.add,
            )
        if self.out_dma is not None:
            nc.sync.dma_start(out=self.out_dma, in_=self.out_sb)


def build_module():
    nc = bass.Bass("TRN2", target_bir_lowering=False, debug=False,
                   enable_asserts=False)
    xoffs = nc.dram_tensor("xoffs", [NPROB, C, L], BF16, kind="ExternalInput").ap()
    xT_d = nc.dram_tensor("xT", [NPROB, 128, NM * C], F32, kind="ExternalInput").ap()
    wqk = nc.dram_tensor("wqk", [C + 1, 40], BF16, kind="ExternalInput").ap()
    wv = nc.dram_tensor("wv", [C + 1, C], BF16, kind="ExternalInput").ap()
    invg_col = nc.dram_tensor("invg_col", [128, NM], BF16, kind="ExternalInput").ap()
    out_d = nc.dram_tensor("out", [NPROB, 128, NM * C], F32, kind="ExternalOutput").ap()

    with tile.TileContext(nc) as tc:
        with (
            tc.tile_pool(name="singles", bufs=1) as singles,
            tc.tile_pool(name="io", bufs=2) as io,
            tc.tile_pool(name="qk", bufs=2) as qkp,
            tc.tile_pool(name="vt", bufs=2) as vtp,
            tc.tile_pool(name="pt", bufs=2) as ptp,
            tc.tile_pool(name="small", bufs=2) as smallp,
            tc.tile_pool(name="ps_s", bufs=3, space="PSUM") as ps_sp,
            tc.tile_pool(name="work", bufs=2, space="PSUM") as workp,
        ):
            wqk_sb = singles.tile([C + 1, 40], BF16)
            nc.sync.dma_start(out=wqk_sb, in_=wqk)
            wv_sb = singles.tile([C + 1, C], BF16)
            nc.sync.dma_start(out=wv_sb, in_=wv)
            invg_sb = singles.tile([128, NM], BF16)
            nc.sync.dma_start(out=invg_sb, in_=invg_col)

            av_q = None

            def drain(n):
                if av_q is not None:
                    av_q.drain(n)

            def emit_load(p):
                x_sb = io.tile([C + 1, L], BF16, tag="x")
                nc.sync.dma_start(out=x_sb[0:C, :], in_=xoffs[p])
                nc.gpsimd.memset(x_sb[C : C + 1, :], 1.0)
                xT_sb = io.tile([128, NM * C], F32, tag="xt")
                nc.sync.dma_start(out=xT_sb, in_=xT_d[p])
                out_sb = io.tile([128, NM * C], F32, tag="out")
                return x_sb, xT_sb, out_sb

            def emit_proj(p, x_sb):
                # q/k projection: one relu per block into qk_sb [40, L];
                # GpSimd copies the k rows to base-0 k0_sb.
                qk_sb = qkp.tile([40, L], BF16, tag="qk")
                k0_sb = qkp.tile([C8, L], BF16, tag="k0")
                for st, w in LBLOCKS:
                    ps = workp.tile([128, 512], F32, tag="work")
                    nc.tensor.matmul(
                        ps[:40, :w], lhsT=wqk_sb, rhs=x_sb[:, st : st + w],
                        start=True, stop=True,
                    )
                    drain(9)
                    nc.vector.tensor_scalar_max(
                        out=qk_sb[:, st : st + w], in0=ps[:40, :w], scalar1=0.0)
                    nc.gpsimd.tensor_copy(
                        k0_sb[:, st : st + w], qk_sb[32:40, st : st + w])
                # v projection, transposed, 128-row chunks
                vT_sb = vtp.tile([128, NM * VS], BF16, tag="vt")
                vT3 = vT_sb.rearrange("p (n c) -> p n c", c=VS)
                nc.gpsimd.tensor_copy(vT3[:, :, C], invg_sb)
                for g in range(3):
                    cnt = 8 if g < 2 else NM - 16
                    ps = workp.tile([128, 512], F32, tag="work")
                    for j in range(cnt):
                        mc = g * 8 + j
                        nc.tensor.matmul(
                            ps[:, j * C : (j + 1) * C],
                            lhsT=x_sb[:, mc * 128 : (mc + 1) * 128],
                            rhs=wv_sb, start=True, stop=True,
                        )
                    drain(9)
                    ps3 = ps.rearrange("p (n c) -> p n c", c=C)
                    nc.scalar.activation(
                        out=vT3[:, g * 8 : g * 8 + cnt, 0:C],
                        in_=ps3[:, 0:cnt, :], func=AF.Relu)
                return qk_sb, k0_sb, vT3

            x_sb, xT_sb, out_sb = emit_load(0)
            qk_sb, k0_sb, vT3 = emit_proj(0, x_sb)
            next_load = None

            for p in range(NPROB):
                for bi, (st, w) in enumerate(LBLOCKS):
                    if bi == 1 and p + 1 < NPROB:
                        next_load = emit_load(p + 1)
                    pT_sb = ptp.tile([128, NM * 512], BF16, tag="pt")
                    pT3 = pT_sb.rearrange("p (n c) -> p n c", c=512)
                    eng = EXP_PATTERNS[bi % len(EXP_PATTERNS)]
                    for g in range(NGRP):
                        ps_s = ps_sp.tile([128, 1024], F32, tag="s")
                        for j in range(2):
                            mc = 2 * g + j
                            nc.tensor.matmul(
                                ps_s[:, j * 512 : j * 512 + w],
                                lhsT=k0_sb[:, mc * 128 : (mc + 1) * 128],
                                rhs=qk_sb[0:C8, st : st + w],
                                start=True, stop=True,
                            )
                        drain(8)
                        ps_s3 = ps_s.rearrange("p (n c) -> p n c", c=512)
                        if eng[g] == "A":
                            nc.scalar.activation(
                                out=pT3[:, 2 * g : 2 * g + 2, :w],
                                in_=ps_s3[:, :, :w], func=AF.Exp)
                        else:
                            nc.vector.tensor_scalar(
                                out=pT3[:, 2 * g : 2 * g + 2, :w].bitcast(I16),
                                in0=ps_s3[:, :, :w], scalar1=A16, scalar2=B16,
                                op0=AT.mult, op1=AT.add)
                    if av_q is not None:
                        av_q.finish()
                    is_last = (st, w) == LBLOCKS[-1]
                    av_q = AvQueue(
                        nc, workp, smallp, pT3, st, w, vT3, out_sb, xT_sb,
                        out_dma=out_d[p] if is_last else None)
                if p + 1 < NPROB:
                    nx, nxT, nout = next_load
                    nqk, nk0, nvT3 = emit_proj(p + 1, nx)
                    x_sb, xT_sb, out_sb = nx, nxT, nout
                    qk_sb, k0_sb, vT3 = nqk, nk0, nvT3
            av_q.finish()

    split_drain_waits(nc)
    return nc


_NC = None


def _get_nc():
    global _NC
    if _NC is None:
        _NC = build_module()
    return _NC


def make_in_maps(x, Wq, bq, Wk, bk, Wv, bv, gamma):
    bf = ml_dtypes.bfloat16
    x = np.asarray(x, np.float32)
    xoff = (
        x.reshape(B, C, HQ, 4, WQ, 4)
        .transpose(0, 3, 5, 1, 2, 4)
        .reshape(B * 16, C, L)
    )
    xoff_bf = np.ascontiguousarray(xoff.astype(bf))
    # transposed residual, chunk-major: [prob, 128, NM*C]
    xT = np.ascontiguousarray(
        xoff.transpose(0, 2, 1)
        .reshape(B * 16, NM, 128, C)
        .transpose(0, 2, 1, 3)
        .reshape(B * 16, 128, NM * C)
    )
    wqk = np.zeros((C + 1, 40), np.float32)   # q -> psum parts 0-7, k -> 32-39
    wqk[:C, 0:C8] = np.asarray(Wq).T
    wqk[C, 0:C8] = np.asarray(bq)
    wqk[:C, 32:40] = np.asarray(Wk).T
    wqk[C, 32:40] = np.asarray(bk)
    wqk = wqk.astype(bf)
    wv = np.concatenate([np.asarray(Wv).T, np.asarray(bv)[None, :]], 0).astype(bf)
    with np.errstate(divide="ignore"):
        invg = np.float32(1.0) / np.float32(np.asarray(gamma).reshape(-1)[0])
    invg_col = np.full((128, NM), invg, np.float32).astype(bf)
    in_maps = []
    for c in range(NCORES):
        sl = slice(c * NPROB, (c + 1) * NPROB)
        in_maps.append(
            {
                "xoffs": np.ascontiguousarray(xoff_bf[sl]),
                "xT": np.ascontiguousarray(xT[sl]),
                "wqk": wqk,
                "wv": wv,
                "invg_col": invg_col,
            }
        )
    return in_maps


def unshard(results):
    outp = np.concatenate([results[c]["out"] for c in range(NCORES)], 0)
    # [32, 128, NM*C] l-minor-transposed -> [32, C, L]
    outp = (
        outp.reshape(B * 16, 128, NM, C)
        .transpose(0, 3, 2, 1)          # [32, C, NM, 128]
        .reshape(B * 16, C, L)
    )
    return (
        outp.reshape(B, 4, 4, C, HQ, WQ)
        .transpose(0, 3, 4, 1, 5, 2)
        .reshape(B, C, H, W)
        .astype(np.float32)
    )


def kernel(**inputs):
    nc = _get_nc()
    in_maps = make_in_maps(**inputs)
    res = run_bass_kernel_spmd(nc, in_maps, list(range(NCORES)))
    return unshard(res.results)


# revision 12
# speedup vs baseline: 1.5111x; 1.0486x over previous
"""ChessBoardAttention Trainium2 kernel.

Full inputs -> full output. The 32 independent (batch, chessboard-offset)
attention problems are sharded 4-per-core across 8 NeuronCores; the
chessboard gather/scatter is pure data movement done host-side as part of
sharding.

Per-core device kernel, per problem (x_off: [64, 2304]), all matmul
operands bf16:
  qk  = relu(Wqk @ x + b)            [40, L]  one relu per l-block
                                     (q rows 0-7, k rows 32-39);
                                     GpSimd copies k to a base-0 tile
  vT  = relu(x_chunk.T @ Wv.T + bv)  [128-chunks, 65]  col 64 = 1/gamma
  S_T[m, l] = k[:,m-chunk].T @ q     scores TRANSPOSED, 2-m-chunk psum groups
  P_T = exp(S_T)                     split between Act (exact Exp) and DVE
                                     (Schraudolph: bf16 bits = rint(a*s+b)
                                     via fp32->int16 convert, bitcast)
  AV (transposed): out_T[l, c] = sum_m P_T[m, l] vT[m, c] accumulated over
      18 m-chunks into PSUM [128, 65]; col 64 = Z/gamma.
  out_T = (out_T[:, :64] * (gamma/Z)[l]) + xT   fused scalar_tensor_tensor
  Output written l-major [128, 18*64]; host undoes the transpose.

The AV matmuls of block b are interleaved between the score-matmul groups
of block b+1 (and the projection groups of the next problem) so the PE
never idles while the exp engines drain score psum groups.
"""

import numpy as np
import ml_dtypes

import concourse.bass as bass
import concourse.tile as tile
from concourse import mybir
from concourse.bass_utils import run_bass_kernel_spmd

F32 = mybir.dt.float32
BF16 = mybir.dt.bfloat16
I16 = mybir.dt.int16
AT = mybir.AluOpType
AF = mybir.ActivationFunctionType

B, C, H, W = 2, 64, 192, 192
C8 = 8
HQ, WQ = H // 4, W // 4
L = HQ * WQ            # 2304
NPROB = 4              # problems per core
NCORES = 8
NM = L // 128          # 18 m-chunks of 128
LBLOCKS = [(0, 512), (512, 512), (1024, 512), (1536, 512), (2048, 256)]
VS = C + 1             # v-chunk stride in vT_sb (64 channels + 1/gamma col)
NGRP = NM // 2         # 9 score psum groups of 2 m-chunks per l-block

# Schraudolph exp for bf16: bits16 = rint(A16*s + B16); bitcast int16->bf16.
A16 = float(128.0 / np.log(2.0))
B16 = float(127.0 * 128.0 - 7.4)

# exp engine per score group, cycled per block: Act ~5.3, DVE ~3.7 of 9
EXP_PATTERNS = [
    ["A", "D", "A", "D", "A", "D", "A", "D", "A"],   # 5A/4D
    ["A", "D", "A", "D", "A", "D", "A", "D", "A"],   # 5A/4D
    ["A", "D", "A", "A", "D", "A", "A", "D", "A"],   # 6A/3D
]


def split_drain_waits(nc, keep=1):
    """This walrus build rejects instructions carrying more than a couple of
    sem-waits. Move excess waits onto single-wait DRAIN instructions inserted
    just before the offender on the same engine (drains with one wait are
    known-good through codegen)."""
    for f in nc.m.functions:
        for bb in f.blocks:
            insts = bb.instructions
            idx = 0
            while idx < len(insts):
                i = insts[idx]
                si = i.sync_info
                lim = keep
                if si is not None and si.on_wait and len(si.on_wait) > lim:
                    waits = list(si.on_wait)
                    si.on_wait = waits[-lim:]
                    for k, wt in enumerate(waits[:-lim]):
                        d = mybir.InstDrain(
                            name=f"{i.name}_wsplit{k}", ins=[], outs=[],
                            bass_is_fusable=False,
                        )
                        d.engine = i.engine
                        d.sync_info = mybir.SyncInfo(on_wait=[wt], on_update=[])
                        nc.register_instruction(d)
                        insts.insert(idx, d)
                        idx += 1
                idx += 1


class AvQueue:
    """Pending AV matmuls for one finished l-block, drained a few at a time
    between later PE work so the tensor engine never stalls on exp."""

    def __init__(self, nc, work_pool, small_pool, pT3, st, w, vT3, out_sb,
                 xT_sb, out_dma=None):
        self.nc = nc
        self.small = small_pool
        self.pT3, self.st, self.w = pT3, st, w
        self.vT3, self.out_sb, self.xT_sb = vT3, out_sb, xT_sb
        self.out_dma = out_dma
        self.nsub = w // 128
        self.ps_av = work_pool.tile([128, 512], F32, tag="work")
        self.items = [(sub, mc) for sub in range(self.nsub) for mc in range(NM)]
        self.pos = 0

    def drain(self, n):
        nc = self.nc
        end = min(self.pos + n, len(self.items))
        for i in range(self.pos, end):
            sub, mc = self.items[i]
            nc.tensor.matmul(
                self.ps_av[:, sub * VS : sub * VS + VS],
                lhsT=self.pT3[:, mc, sub * 128 : (sub + 1) * 128],
                rhs=self.vT3[:, mc, :],
                start=(mc == 0), stop=(mc == NM - 1),
            )
        self.pos = end

    def finish(self):
        nc = self.nc
        self.drain(len(self.items))
        rec = self.small.tile([128, 4], F32, tag="rec")
        zview = bass.AP(
            tensor=self.ps_av.tensor, offset=self.ps_av.offset + C,
            ap=[list(self.ps_av.ap)[0], [VS, self.nsub]])
        nc.vector.reciprocal(out=rec[:, 0 : self.nsub], in_=zview)
        for sub in range(self.nsub):
            ci = self.st // 128 + sub
            nc.vector.scalar_tensor_tensor(
                out=self.out_sb[:, ci * C : (ci + 1) * C],
                in0=self.ps_av[:, sub * VS : sub * VS + C],
                scalar=rec[:, sub : sub + 1],
                in1=self.xT_sb[:, ci * C : (ci + 1) * C],
                op0=AT.mult, op1=AT.add,
            )
        if self.out_dma is not None:
            nc.sync.dma_start(out=self.out_dma, in_=self.out_sb)


def build_module():
    nc = bass.Bass("TRN2", target_bir_lowering=False, debug=False,
                   enable_asserts=False)
    xoffs = nc.dram_tensor("xoffs", [NPROB, C, L], BF16, kind="ExternalInput").ap()
    xT_d = nc.dram_tensor("xT", [NPROB, 128, NM * C], F32, kind="ExternalInput").ap()
    wqk = nc.dram_tensor("wqk", [C + 1, 40], BF16, kind="ExternalInput").ap()
    wv = nc.dram_tensor("wv", [C + 1, C], BF16, kind="ExternalInput").ap()
    invg_col = nc.dram_tensor("invg_col", [128, NM], BF16, kind="ExternalInput").ap()
    out_d = nc.dram_tensor("out", [NPROB, 128, NM * C], F32, kind="ExternalOutput").ap()

    with tile.TileContext(nc) as tc:
        with (
            tc.tile_pool(name="singles", bufs=1) as singles,
            tc.tile_pool(name="io", bufs=2) as io,
            tc.tile_pool(name="qk", bufs=2) as qkp,
            tc.tile_pool(name="vt", bufs=2) as vtp,
            tc.tile_pool(name="pt", bufs=2) as ptp,
            tc.tile_pool(name="small", bufs=2) as smallp,
            tc.tile_pool(name="ps_s", bufs=3, space="PSUM") as ps_sp,
            tc.tile_pool(name="work", bufs=2, space="PSUM") as workp,
        ):
            wqk_sb = singles.tile([C + 1, 40], BF16)
            nc.sync.dma_start(out=wqk_sb, in_=wqk)
            wv_sb = singles.tile([C + 1, C], BF16)
            nc.sync.dma_start(out=wv_sb, in_=wv)
            invg_sb = singles.tile([128, NM], BF16)
            nc.sync.dma_start(out=invg_sb, in_=invg_col)

            av_q = None

            def drain(n):
                if av_q is not None:
                    av_q.drain(n)

            def emit_load(p):
                x_sb = io.tile([C + 1, L], BF16, tag="x")
                for st, w in LBLOCKS:
                    nc.sync.dma_start(
                        out=x_sb[0:C, st : st + w], in_=xoffs[p][:, st : st + w])
                nc.gpsimd.memset(x_sb[C : C + 1, :], 1.0)
                xT_sb = io.tile([128, NM * C], F32, tag="xt")
                nc.sync.dma_start(out=xT_sb, in_=xT_d[p])
                out_sb = io.tile([128, NM * C], F32, tag="out")
                return x_sb, xT_sb, out_sb

            def emit_proj(p, x_sb):
                # q/k projection: one relu per block into qk_sb [40, L];
                # GpSimd copies the k rows to base-0 k0_sb.
                qk_sb = qkp.tile([40, L], BF16, tag="qk")
                k0_sb = qkp.tile([C8, L], BF16, tag="k0")
                for st, w in LBLOCKS:
                    ps = workp.tile([128, 512], F32, tag="work")
                    nc.tensor.matmul(
                        ps[:40, :w], lhsT=wqk_sb, rhs=x_sb[:, st : st + w],
                        start=True, stop=True,
                    )
                    drain(4)
                    nc.scalar.activation(
                        out=qk_sb[:, st : st + w], in_=ps[:40, :w], func=AF.Relu)
                    nc.gpsimd.tensor_copy(
                        k0_sb[:, st : st + w], qk_sb[32:40, st : st + w])
                # v projection, transposed, 128-row chunks
                vT_sb = vtp.tile([128, NM * VS], BF16, tag="vt")
                vT3 = vT_sb.rearrange("p (n c) -> p n c", c=VS)
                nc.gpsimd.tensor_copy(vT3[:, :, C], invg_sb)
                for g in range(3):
                    cnt = 8 if g < 2 else NM - 16
                    ps = workp.tile([128, 512], F32, tag="work")
                    for j in range(cnt):
                        mc = g * 8 + j
                        nc.tensor.matmul(
                            ps[:, j * C : (j + 1) * C],
                            lhsT=x_sb[:, mc * 128 : (mc + 1) * 128],
                            rhs=wv_sb, start=True, stop=True,
                        )
                    drain(4)
                    ps3 = ps.rearrange("p (n c) -> p n c", c=C)
                    nc.vector.tensor_scalar_max(
                        out=vT3[:, g * 8 : g * 8 + cnt, 0:C],
                        in0=ps3[:, 0:cnt, :], scalar1=0.0)
                return qk_sb, k0_sb, vT3

            x_sb, xT_sb, out_sb = emit_load(0)
            qk_sb, k0_sb, vT3 = emit_proj(0, x_sb)
            next_load = None

            for p in range(NPROB):
                for bi, (st, w) in enumerate(LBLOCKS):
                    if bi == 1 and p + 1 < NPROB:
                        next_load = emit_load(p + 1)
                    pT_sb = ptp.tile([128, NM * 512], BF16, tag="pt")
                    pT3 = pT_sb.rearrange("p (n c) -> p n c", c=512)
                    eng = EXP_PATTERNS[bi % len(EXP_PATTERNS)]
                    for g in range(NGRP):
                        ps_s = ps_sp.tile([128, 1024], F32, tag="s")
                        for j in range(2):
                            mc = 2 * g + j
                            nc.tensor.matmul(
                                ps_s[:, j * 512 : j * 512 + w],
                                lhsT=k0_sb[:, mc * 128 : (mc + 1) * 128],
                                rhs=qk_sb[0:C8, st : st + w],
                                start=True, stop=True,
                            )
                        drain(8)
                        ps_s3 = ps_s.rearrange("p (n c) -> p n c", c=512)
                        if eng[g] == "A":
                            nc.scalar.activation(
                                out=pT3[:, 2 * g : 2 * g + 2, :w],
                                in_=ps_s3[:, :, :w], func=AF.Exp)
                        else:
                            nc.vector.tensor_scalar(
                                out=pT3[:, 2 * g : 2 * g + 2, :w].bitcast(I16),
                                in0=ps_s3[:, :, :w], scalar1=A16, scalar2=B16,
                                op0=AT.mult, op1=AT.add)
                    if av_q is not None:
                        av_q.finish()
                    is_last = (st, w) == LBLOCKS[-1]
                    av_q = AvQueue(
                        nc, workp, smallp, pT3, st, w, vT3, out_sb, xT_sb,
                        out_dma=out_d[p] if is_last else None)
                if p + 1 < NPROB:
                    nx, nxT, nout = next_load
                    nqk, nk0, nvT3 = emit_proj(p + 1, nx)
                    x_sb, xT_sb, out_sb = nx, nxT, nout
                    qk_sb, k0_sb, vT3 = nqk, nk0, nvT3
            av_q.finish()

    split_drain_waits(nc)
    return nc


_NC = None


def _get_nc():
    global _NC
    if _NC is None:
        _NC = build_module()
    return _NC


def make_in_maps(x, Wq, bq, Wk, bk, Wv, bv, gamma):
    bf = ml_dtypes.bfloat16
    x = np.asarray(x, np.float32)
    xoff = (
        x.reshape(B, C, HQ, 4, WQ, 4)
        .transpose(0, 3, 5, 1, 2, 4)
        .reshape(B * 16, C, L)
    )
    xoff_bf = np.ascontiguousarray(xoff.astype(bf))
    # transposed residual, chunk-major: [prob, 128, NM*C]
    xT = np.ascontiguousarray(
        xoff.transpose(0, 2, 1)
        .reshape(B * 16, NM, 128, C)
        .transpose(0, 2, 1, 3)
        .reshape(B * 16, 128, NM * C)
    )
    wqk = np.zeros((C + 1, 40), np.float32)   # q -> psum parts 0-7, k -> 32-39
    wqk[:C, 0:C8] = np.asarray(Wq).T
    wqk[C, 0:C8] = np.asarray(bq)
    wqk[:C, 32:40] = np.asarray(Wk).T
    wqk[C, 32:40] = np.asarray(bk)
    wqk = wqk.astype(bf)
    wv = np.concatenate([np.asarray(Wv).T, np.asarray(bv)[None, :]], 0).astype(bf)
    with np.errstate(divide="ignore"):
        invg = np.float32(1.0) / np.float32(np.asarray(gamma).reshape(-1)[0])
    invg_col = np.full((128, NM), invg, np.float32).astype(bf)
    in_maps = []
    for c in range(NCORES):
        sl = slice(c * NPROB, (c + 1) * NPROB)
        in_maps.append(
            {
                "xoffs": np.ascontiguousarray(xoff_bf[sl]),
                "xT": np.ascontiguousarray(xT[sl]),
                "wqk": wqk,
                "wv": wv,
                "invg_col": invg_col,
            }
        )
    return in_maps


def unshard(results):
    outp = np.concatenate([results[c]["out"] for c in range(NCORES)], 0)
    # [32, 128, NM*C] l-minor-transposed -> [32, C, L]
    outp = (
        outp.reshape(B * 16, 128, NM, C)
        .transpose(0, 3, 2, 1)          # [32, C, NM, 128]
        .reshape(B * 16, C, L)
    )
    return (
        outp.reshape(B, 4, 4, C, HQ, WQ)
        .transpose(0, 3, 4, 1, 5, 2)
        .reshape(B, C, H, W)
        .astype(np.float32)
    )


def kernel(**inputs):
    nc = _get_nc()
    in_maps = make_in_maps(**inputs)
    res = run_bass_kernel_spmd(nc, in_maps, list(range(NCORES)))
    return unshard(res.results)


# revision 14
# speedup vs baseline: 1.5118x; 1.0004x over previous
"""ChessBoardAttention Trainium2 kernel.

Full inputs -> full output. The 32 independent (batch, chessboard-offset)
attention problems are sharded 4-per-core across 8 NeuronCores; the
chessboard gather/scatter is pure data movement done host-side as part of
sharding.

Per-core device kernel, per problem (x_off: [64, 2304]), all matmul
operands bf16:
  qk  = relu(Wqk @ x + b)            [40, L]  one relu per l-block
                                     (q rows 0-7, k rows 32-39);
                                     GpSimd copies k to a base-0 tile
  vT  = relu(x_chunk.T @ Wv.T + bv)  [128-chunks, 65]  col 64 = 1/gamma
  S_T[m, l] = k[:,m-chunk].T @ q     scores TRANSPOSED, 2-m-chunk psum groups
  P_T = exp(S_T)                     split between Act (exact Exp) and DVE
                                     (Schraudolph: bf16 bits = rint(a*s+b)
                                     via fp32->int16 convert, bitcast)
  AV (transposed): out_T[l, c] = sum_m P_T[m, l] vT[m, c] accumulated over
      18 m-chunks into PSUM [128, 65]; col 64 = Z/gamma.
  out_T = (out_T[:, :64] * (gamma/Z)[l]) + xT   fused scalar_tensor_tensor
  Output written l-major [128, 18*64]; host undoes the transpose.

The AV matmuls of block b are interleaved between the score-matmul groups
of block b+1 (and the projection groups of the next problem) so the PE
never idles while the exp engines drain score psum groups.
"""

import numpy as np
import ml_dtypes

import concourse.bass as bass
import concourse.tile as tile
from concourse import mybir
from concourse.bass_utils import run_bass_kernel_spmd

F32 = mybir.dt.float32
BF16 = mybir.dt.bfloat16
I16 = mybir.dt.int16
AT = mybir.AluOpType
AF = mybir.ActivationFunctionType

B, C, H, W = 2, 64, 192, 192
C8 = 8
HQ, WQ = H // 4, W // 4
L = HQ * WQ            # 2304
NPROB = 4              # problems per core
NCORES = 8
NM = L // 128          # 18 m-chunks of 128
LBLOCKS = [(0, 512), (512, 512), (1024, 512), (1536, 512), (2048, 256)]
VS = C + 1             # v-chunk stride in vT_sb (64 channels + 1/gamma col)
NGRP = NM // 2         # 9 score psum groups of 2 m-chunks per l-block

# Schraudolph exp for bf16: bits16 = rint(A16*s + B16); bitcast int16->bf16.
A16 = float(128.0 / np.log(2.0))
B16 = float(127.0 * 128.0 - 7.4)

# exp engine per score group, cycled per block: Act ~5.3, DVE ~3.7 of 9
EXP_PATTERNS = [
    ["A", "D", "A", "D", "A", "D", "A", "D", "A"],   # 5A/4D
    ["A", "D", "A", "D", "A", "D", "A", "D", "A"],   # 5A/4D
    ["A", "D", "A", "A", "D", "A", "A", "D", "A"],   # 6A/3D
]


def split_drain_waits(nc, keep=1):
    """This walrus build rejects instructions carrying more than a couple of
    sem-waits. Move excess waits onto single-wait DRAIN instructions inserted
    just before the offender on the same engine (drains with one wait are
    known-good through codegen)."""
    for f in nc.m.functions:
        for bb in f.blocks:
            insts = bb.instructions
            idx = 0
            while idx < len(insts):
                i = insts[idx]
                si = i.sync_info
                lim = keep
                if si is not None and si.on_wait and len(si.on_wait) > lim:
                    waits = list(si.on_wait)
                    si.on_wait = waits[-lim:]
                    for k, wt in enumerate(waits[:-lim]):
                        d = mybir.InstDrain(
                            name=f"{i.name}_wsplit{k}", ins=[], outs=[],
                            bass_is_fusable=False,
                        )
                        d.engine = i.engine
                        d.sync_info = mybir.SyncInfo(on_wait=[wt], on_update=[])
                        nc.register_instruction(d)
                        insts.insert(idx, d)
                        idx += 1
                idx += 1


class AvQueue:
    """Pending AV matmuls for one finished l-block, drained a few at a time
    between later PE work so the tensor engine never stalls on exp."""

    def __init__(self, nc, work_pool, small_pool, pT3, st, w, vT3, out_sb,
                 xT_sb, out_dma=None):
        self.nc = nc
        self.small = small_pool
        self.pT3, self.st, self.w = pT3, st, w
        self.vT3, self.out_sb, self.xT_sb = vT3, out_sb, xT_sb
        self.out_dma = out_dma
        self.nsub = w // 128
        self.ps_av = work_pool.tile([128, 512], F32, tag="work")
        self.items = [(sub, mc) for sub in range(self.nsub) for mc in range(NM)]
        self.pos = 0

    def drain(self, n):
        nc = self.nc
        end = min(self.pos + n, len(self.items))
        for i in range(self.pos, end):
            sub, mc = self.items[i]
            nc.tensor.matmul(
                self.ps_av[:, sub * VS : sub * VS + VS],
                lhsT=self.pT3[:, mc, sub * 128 : (sub + 1) * 128],
                rhs=self.vT3[:, mc, :],
                start=(mc == 0), stop=(mc == NM - 1),
            )
        self.pos = end

    def finish(self):
        nc = self.nc
        self.drain(len(self.items))
        rec = self.small.tile([128, 4], F32, tag="rec")
        zview = bass.AP(
            tensor=self.ps_av.tensor, offset=self.ps_av.offset + C,
            ap=[list(self.ps_av.ap)[0], [VS, self.nsub]])
        nc.vector.reciprocal(out=rec[:, 0 : self.nsub], in_=zview)
        for sub in range(self.nsub):
            ci = self.st // 128 + sub
            nc.vector.scalar_tensor_tensor(
                out=self.out_sb[:, ci * C : (ci + 1) * C],
                in0=self.ps_av[:, sub * VS : sub * VS + C],
                scalar=rec[:, sub : sub + 1],
                in1=self.xT_sb[:, ci * C : (ci + 1) * C],
                op0=AT.mult, op1=AT.add,
            )
        if self.out_dma is not None:
            nc.sync.dma_start(out=self.out_dma, in_=self.out_sb)


def build_module():
    nc = bass.Bass("TRN2", target_bir_lowering=False, debug=False,
                   enable_asserts=False)
    xoffs = nc.dram_tensor("xoffs", [NPROB, C, L], BF16, kind="ExternalInput").ap()
    xT_d = nc.dram_tensor("xT", [NPROB, 128, NM * C], F32, kind="ExternalInput").ap()
    wqk = nc.dram_tensor("wqk", [C + 1, 40], BF16, kind="ExternalInput").ap()
    wv = nc.dram_tensor("wv", [C + 1, C], BF16, kind="ExternalInput").ap()
    invg_col = nc.dram_tensor("invg_col", [128, NM], BF16, kind="ExternalInput").ap()
    out_d = nc.dram_tensor("out", [NPROB, 128, NM * C], F32, kind="ExternalOutput").ap()

    with tile.TileContext(nc) as tc:
        with (
            tc.tile_pool(name="singles", bufs=1) as singles,
            tc.tile_pool(name="io", bufs=2) as io,
            tc.tile_pool(name="qk", bufs=2) as qkp,
            tc.tile_pool(name="vt", bufs=2) as vtp,
            tc.tile_pool(name="pt", bufs=2) as ptp,
            tc.tile_pool(name="small", bufs=2) as smallp,
            tc.tile_pool(name="ps_s", bufs=3, space="PSUM") as ps_sp,
            tc.tile_pool(name="work", bufs=2, space="PSUM") as workp,
        ):
            wqk_sb = singles.tile([C + 1, 40], BF16)
            nc.sync.dma_start(out=wqk_sb, in_=wqk)
            wv_sb = singles.tile([C + 1, C], BF16)
            nc.sync.dma_start(out=wv_sb, in_=wv)
            invg_sb = singles.tile([128, NM], BF16)
            nc.sync.dma_start(out=invg_sb, in_=invg_col)

            av_q = None

            def drain(n):
                if av_q is not None:
                    av_q.drain(n)

            def emit_load(p):
                x_sb = io.tile([C + 1, L], BF16, tag="x")
                for st, w in LBLOCKS:
                    nc.sync.dma_start(
                        out=x_sb[0:C, st : st + w], in_=xoffs[p][:, st : st + w])
                nc.gpsimd.memset(x_sb[C : C + 1, :], 1.0)
                xT_sb = io.tile([128, NM * C], F32, tag="xt")
                nc.sync.dma_start(out=xT_sb, in_=xT_d[p])
                out_sb = io.tile([128, NM * C], F32, tag="out")
                return x_sb, xT_sb, out_sb

            def make_proj_tasks(p, x_sb, sink):
                """Projection for problem p as slot-sized tasks. Each task is
                one psum group: a few PE matmuls + one relu (+ k copy)."""
                qk_sb = qkp.tile([40, L], BF16, tag="qk")
                k0_sb = qkp.tile([C8, L], BF16, tag="k0")
                vT_sb = vtp.tile([128, NM * VS], BF16, tag="vt")
                vT3 = vT_sb.rearrange("p (n c) -> p n c", c=VS)
                sink.update(qk=qk_sb, k0=k0_sb, vT3=vT3)

                def qk_task(st, w):
                    def run():
                        ps = workp.tile([128, 512], F32, tag="work")
                        nc.tensor.matmul(
                            ps[:40, :w], lhsT=wqk_sb, rhs=x_sb[:, st : st + w],
                            start=True, stop=True,
                        )
                        nc.scalar.activation(
                            out=qk_sb[:, st : st + w], in_=ps[:40, :w],
                            func=AF.Relu)
                        nc.gpsimd.tensor_copy(
                            k0_sb[:, st : st + w], qk_sb[32:40, st : st + w])
                    return run

                def v_task(g):
                    def run():
                        if g == 0:
                            nc.gpsimd.tensor_copy(vT3[:, :, C], invg_sb)
                        cnt = 8 if g < 2 else NM - 16
                        ps = workp.tile([128, 512], F32, tag="work")
                        for j in range(cnt):
                            mc = g * 8 + j
                            nc.tensor.matmul(
                                ps[:, j * C : (j + 1) * C],
                                lhsT=x_sb[:, mc * 128 : (mc + 1) * 128],
                                rhs=wv_sb, start=True, stop=True,
                            )
                        ps3 = ps.rearrange("p (n c) -> p n c", c=C)
                        nc.vector.tensor_scalar_max(
                            out=vT3[:, g * 8 : g * 8 + cnt, 0:C],
                            in0=ps3[:, 0:cnt, :], scalar1=0.0)
                    return run

                return [qk_task(st, w) for st, w in LBLOCKS] + \
                       [v_task(g) for g in range(3)]

            x_sb, xT_sb, out_sb = emit_load(0)
            sink0 = {}
            for t in make_proj_tasks(0, x_sb, sink0):
                t()
            qk_sb, k0_sb, vT3 = sink0["qk"], sink0["k0"], sink0["vT3"]
            next_load = None
            nsink = {}
            pending = []

            for p in range(NPROB):
                for bi, (st, w) in enumerate(LBLOCKS):
                    if bi == 1 and p + 1 < NPROB:
                        next_load = emit_load(p + 1)
                    if bi == 3 and p + 1 < NPROB:
                        nsink = {}
                        pending = make_proj_tasks(p + 1, next_load[0], nsink)
                    pT_sb = ptp.tile([128, NM * 512], BF16, tag="pt")
                    pT3 = pT_sb.rearrange("p (n c) -> p n c", c=512)
                    eng = EXP_PATTERNS[bi % len(EXP_PATTERNS)]
                    for g in range(NGRP):
                        ps_s = ps_sp.tile([128, 1024], F32, tag="s")
                        for j in range(2):
                            mc = 2 * g + j
                            nc.tensor.matmul(
                                ps_s[:, j * 512 : j * 512 + w],
                                lhsT=k0_sb[:, mc * 128 : (mc + 1) * 128],
                                rhs=qk_sb[0:C8, st : st + w],
                                start=True, stop=True,
                            )
                        drain(8)
                        ps_s3 = ps_s.rearrange("p (n c) -> p n c", c=512)
                        if eng[g] == "A":
                            nc.scalar.activation(
                                out=pT3[:, 2 * g : 2 * g + 2, :w],
                                in_=ps_s3[:, :, :w], func=AF.Exp)
                        else:
                            nc.vector.tensor_scalar(
                                out=pT3[:, 2 * g : 2 * g + 2, :w].bitcast(I16),
                                in0=ps_s3[:, :, :w], scalar1=A16, scalar2=B16,
                                op0=AT.mult, op1=AT.add)
                        if pending:
                            pending.pop(0)()
                    if av_q is not None:
                        av_q.finish()
                    is_last = (st, w) == LBLOCKS[-1]
                    av_q = AvQueue(
                        nc, workp, smallp, pT3, st, w, vT3, out_sb, xT_sb,
                        out_dma=out_d[p] if is_last else None)
                if p + 1 < NPROB:
                    x_sb, xT_sb, out_sb = next_load
                    qk_sb, k0_sb, vT3 = nsink["qk"], nsink["k0"], nsink["vT3"]
            av_q.finish()

    split_drain_waits(nc)
    return nc


_NC = None


def _get_nc():
    global _NC
    if _NC is None:
        _NC = build_module()
    return _NC


def make_in_maps(x, Wq, bq, Wk, bk, Wv, bv, gamma):
    bf = ml_dtypes.bfloat16
    x = np.asarray(x, np.float32)
    xoff = (
        x.reshape(B, C, HQ, 4, WQ, 4)
        .transpose(0, 3, 5, 1, 2, 4)
        .reshape(B * 16, C, L)
    )
    xoff_bf = np.ascontiguousarray(xoff.astype(bf))
    # transposed residual, chunk-major: [prob, 128, NM*C]
    xT = np.ascontiguousarray(
        xoff.transpose(0, 2, 1)
        .reshape(B * 16, NM, 128, C)
        .transpose(0, 2, 1, 3)
        .reshape(B * 16, 128, NM * C)
    )
    wqk = np.zeros((C + 1, 40), np.float32)   # q -> psum parts 0-7, k -> 32-39
    wqk[:C, 0:C8] = np.asarray(Wq).T
    wqk[C, 0:C8] = np.asarray(bq)
    wqk[:C, 32:40] = np.asarray(Wk).T
    wqk[C, 32:40] = np.asarray(bk)
    wqk = wqk.astype(bf)
    wv = np.concatenate([np.asarray(Wv).T, np.asarray(bv)[None, :]], 0).astype(bf)
    with np.errstate(divide="ignore"):
        invg = np.float32(1.0) / np.float32(np.asarray(gamma).reshape(-1)[0])
    invg_col = np.full((128, NM), invg, np.float32).astype(bf)
    in_maps = []
    for c in range(NCORES):
        sl = slice(c * NPROB, (c + 1) * NPROB)
        in_maps.append(
            {
                "xoffs": np.ascontiguousarray(xoff_bf[sl]),
                "xT": np.ascontiguousarray(xT[sl]),
                "wqk": wqk,
                "wv": wv,
                "invg_col": invg_col,
            }
        )
    return in_maps


def unshard(results):
    outp = np.concatenate([results[c]["out"] for c in range(NCORES)], 0)
    # [32, 128, NM*C] l-minor-transposed -> [32, C, L]
    outp = (
        outp.reshape(B * 16, 128, NM, C)
        .transpose(0, 3, 2, 1)          # [32, C, NM, 128]
        .reshape(B * 16, C, L)
    )
    return (
        outp.reshape(B, 4, 4, C, HQ, WQ)
        .transpose(0, 3, 4, 1, 5, 2)
        .reshape(B, C, H, W)
        .astype(np.float32)
    )


def kernel(**inputs):
    nc = _get_nc()
    in_maps = make_in_maps(**inputs)
    res = run_bass_kernel_spmd(nc, in_maps, list(range(NCORES)))
    return unshard(res.results)
